# revision 7
# baseline (speedup 1.0000x reference)
"""Multi-head self-attention, ET-direct variant: computes E-transposed
directly so no PE transposes / PSUM->SBUF copies of E are needed.

Math per head: h = x @ W.T + b; s = h h^T; attn = softmax(s); out = attn @ x.
Softmax is invariant to any per-row bias; we use bias_q = s_qq + delta_q
(delta = f8 rounding of the aug term), still an exact softmax.

ET[m, q] = exp(s_mq - d_q) is computed with m on partitions and q on the
free axis; the per-q bias -d_q enters through an augmented contraction:
an extra rank-1 matmul term lhs[aug, m] = 64, rhs[aug, q] = -d_q/64 (f8).
d_q is extracted from the diagonal scores blocks (computed first), the
attention diagonal is forced to 0 pre-exp (subtract 1e9 on the diagonal),
the AV matmul accumulates I @ x_f16 for the (exactly 1.0) diagonal weight,
and the row-sum Z gets +1. Z is computed by ones @ ET matmuls.

All big matmuls are fp8e4 + DoubleRow (PE peak rate). Queries are
processed in 8 groups of 512 (4 blocks of 128); group g's AV runs
interleaved into group g+1's score stream so the PE never idles while
the Activation engine exponentiates.
"""
import numpy as np
from contextlib import ExitStack

N, D, H = 4096, 512, 8
P = 128
NB = N // P          # 32 n-blocks
DB = D // P          # 4 d-chunks
G = 8                # query groups of 512
NT = 16              # score tiles per group (2 m-blocks each)
N_CORES = 8

_CACHE = {}


def _build(reps: int = 1):
    from concourse import bacc, tile, mybir, masks

    dt = mybir.dt
    f32, f32r, f16 = dt.float32, dt.float32r, dt.float16
    f8 = dt.float8e4
    DR = mybir.MatmulPerfMode.DoubleRow
    AF = mybir.ActivationFunctionType
    ALU = mybir.AluOpType

    nc = bacc.Bacc("TRN2", target_bir_lowering=False, debug=False)

    X = nc.dram_tensor("x", [N, D], f32, kind="ExternalInput")
    W = nc.dram_tensor("w", [D, D], f32, kind="ExternalInput")
    B = nc.dram_tensor("b", [D, 1], f32, kind="ExternalInput")
    OUT = nc.dram_tensor("out", [N, D], f32, kind="ExternalOutput")

    with tile.TileContext(nc) as tc, ExitStack() as ctx:
        # ---- persistent pools -------------------------------------------
        const_pool = ctx.enter_context(tc.tile_pool(name="const", bufs=1))
        x_pool = ctx.enter_context(tc.tile_pool(name="x", bufs=1))
        hT_pool = ctx.enter_context(tc.tile_pool(name="hT", bufs=1))

        ident = const_pool.tile([P, P], f32)
        masks.make_identity(nc, ident[:])
        ident_h = const_pool.tile([P, P], f16)
        nc.vector.tensor_copy(ident_h[:], ident[:])
        # diag-kill pair: (-224*I).T @ I adds -224 on the diagonal, pushing
        # exp(diag) to 0 (the aug-bias f8 rounding is at most +-32, so the
        # diagonal lands below -190). -224 stays finite in float8e4, whose
        # exp=1111 encodings are inf/NaN (max finite 240). DoubleRow shape
        # with a zero second k-tile.
        ident_8 = const_pool.tile([P, 2, P], f8)
        nc.gpsimd.memset(ident_8[:], 0.0)
        nc.vector.tensor_copy(ident_8[:, 0, :], ident[:])
        negI8 = const_pool.tile([P, 2, P], f8)
        nc.gpsimd.memset(negI8[:], 0.0)
        nc.vector.tensor_scalar(negI8[:, 0, :], ident[:], -224.0, None, op0=ALU.mult)
        ones = const_pool.tile([P, 1], f32)
        nc.gpsimd.memset(ones[:], 1.0)
        ones_r = const_pool.tile([P, 1], f32r)
        nc.vector.tensor_copy(ones_r[:], ones[:])
        ones8 = const_pool.tile([P, 2, P], f8)
        nc.gpsimd.memset(ones8[:], 1.0)
        const64 = const_pool.tile([1, 2, P], f8)
        nc.gpsimd.memset(const64[:], 64.0)
        b_sb = const_pool.tile([P, DB], f32)
        for ob in range(DB):
            nc.sync.dma_start(b_sb[:, ob : ob + 1], B.ap()[ob * P : (ob + 1) * P, :])

        # x natural layout: x_sb[p, j, d] = x[j*128 + p, d]
        x_sb = x_pool.tile([P, NB, D], f16)
        x8 = x_pool.tile([P, NB, D], f8)

        # hT[p, dc, n] = h[n, dc*128 + p]
        hT = hT_pool.tile([P, DB, N], f8)

        for rep in range(reps):
            # ---- phase 2 pools (created first: PSUM is shared with phase 1,
            # whose h-matmuls borrow sc-pool tiles and transposes the sm pool;
            # pools are LIFO so phase-1 pools must sit above them) ----------
            p2 = ctx.enter_context(ExitStack()) if reps == 1 else ExitStack()
            E_pool = p2.enter_context(tc.tile_pool(name=f"E{rep}", bufs=2))
            st_pool = p2.enter_context(tc.tile_pool(name=f"st{rep}", bufs=3))
            out_pool = p2.enter_context(tc.tile_pool(name=f"outp{rep}", bufs=3))
            sc_ps_pool = p2.enter_context(
                tc.tile_pool(name=f"scps{rep}", bufs=2, space="PSUM")
            )
            sm_ps_pool = p2.enter_context(
                tc.tile_pool(name=f"smps{rep}", bufs=1, space="PSUM")
            )
            o_ps_pool = p2.enter_context(
                tc.tile_pool(name=f"ops{rep}", bufs=2, space="PSUM")
            )

            # ---- phase 1: hT = (x @ W.T + b).T ------------------------------
            # (emission is interleaved with phase-2 group 0; see driver below)
            p1 = ExitStack()
            w_pool = p1.enter_context(tc.tile_pool(name=f"wp{rep}", bufs=1))
            xT_pool = p1.enter_context(tc.tile_pool(name=f"xTp{rep}", bufs=2))
            xf_pool = p1.enter_context(tc.tile_pool(name=f"xf{rep}", bufs=4))

            def load_x_pair(jp, xT, slot):
                # load + convert x row-blocks 2jp, 2jp+1
                xf = xf_pool.tile([P, 2, D], f32, tag="xf")
                for u in range(2):
                    nc.sync.dma_start(
                        xf[:, u, :],
                        X.ap()[(2 * jp + u) * P : (2 * jp + u + 1) * P, :],
                    )
                nc.gpsimd.tensor_copy(x_sb[:, 2 * jp : 2 * jp + 2, :], xf[:])
                nc.vector.tensor_copy(x8[:, 2 * jp : 2 * jp + 2, :], xf[:])
                for u in range(2):
                    j = 2 * jp + u
                    tp = sm_ps_pool.tile([P, DB, P], f16, tag="tr")
                    for dc in range(DB):
                        nc.tensor.transpose(
                            tp[:, dc, :],
                            x_sb[:, j, dc * P : (dc + 1) * P],
                            ident_h[:],
                        )
                    nc.vector.tensor_copy(
                        xT[:, :, (2 * slot + u) * P : (2 * slot + u + 1) * P],
                        tp[:],
                    )

            def emit_w_pipeline():
                w_f32 = w_pool.tile([P, DB, D], f32)
                for ob in range(DB):
                    nc.sync.dma_start(
                        w_f32[:, ob, :], W.ap()[ob * P : (ob + 1) * P, :]
                    )
                w_sb = w_pool.tile([P, DB, D], f16)
                for ob in range(DB):
                    nc.vector.tensor_copy(w_sb[:, ob, :], w_f32[:, ob, :])
                wT = w_pool.tile([P, DB, D], f8)
                for ob in range(DB):
                    tp = sm_ps_pool.tile([P, DB, P], f16, tag="tr")
                    for dc in range(DB):
                        nc.tensor.transpose(
                            tp[:, dc, :],
                            w_sb[:, ob, dc * P : (dc + 1) * P],
                            ident_h[:],
                        )
                    nc.vector.tensor_copy(wT[:, :, ob * P : (ob + 1) * P], tp[:])
                return wT

            wT_box = [None]

            def p1_chunk(nc512):
                lo, hi = nc512 * 512, (nc512 + 1) * 512
                xT = xT_pool.tile([P, DB, 512], f8, tag="xT")
                for jp in range(2):
                    load_x_pair(nc512 * 2 + jp, xT, jp)
                if nc512 == 0:
                    wT_box[0] = emit_w_pipeline()
                wT = wT_box[0]
                for obp in range(DB // 2):
                    hp = sc_ps_pool.tile([P, 2, 512], f32, tag="s")
                    for t in range(2):
                        ob = 2 * obp + t
                        for c in range(DB // 2):
                            nc.tensor.matmul(
                                hp[:, t, :],
                                wT[:, 2 * c : 2 * c + 2, ob * P : (ob + 1) * P],
                                xT[:, 2 * c : 2 * c + 2, :],
                                start=(c == 0),
                                stop=(c == DB // 2 - 1),
                                perf_mode=DR,
                            )
                    for t in range(2):
                        ob = 2 * obp + t
                        nc.scalar.activation(
                            hT[:, ob, lo:hi],
                            hp[:, t, :],
                            AF.Identity,
                            bias=b_sb[:, ob : ob + 1],
                            scale=1.0,
                        )

            state = {}

            def score_tile_mms(s_ps, j, qlo, g, aug):
                """One accumulation group per m-block: d-contraction pairs,
                the aug bias term, and (diagonal blocks) the -448*I kill."""
                for t in range(2):
                    mb = 2 * j + t
                    for k in range(DB // 2):
                        nc.tensor.matmul(
                            s_ps[:, t, :],
                            hT[:, 2 * k : 2 * k + 2, mb * P : (mb + 1) * P],
                            hT[:, 2 * k : 2 * k + 2, qlo : qlo + 512],
                            start=(k == 0),
                            stop=False,
                            perf_mode=DR,
                        )
                    if 4 * g <= mb < 4 * g + 4:
                        off = (mb - 4 * g) * P
                        nc.tensor.matmul(
                            s_ps[:, t, off : off + P],
                            negI8[:],
                            ident_8[:],
                            start=False,
                            stop=False,
                            perf_mode=DR,
                        )
                    nc.tensor.matmul(
                        s_ps[:, t, :],
                        const64[:],
                        aug[:],
                        start=False,
                        stop=True,
                        perf_mode=DR,
                    )

            _DONE = object()

            def pull(av_iter, k):
                if av_iter is None:
                    return
                for _ in range(k):
                    if next(av_iter, _DONE) is _DONE:
                        return

            def scores_stage(g, av_iter):
                """Generator: emits group g's score/exp/Z stream, yielding the
                hT chunk index it needs next (so group 0 can interleave with
                phase 1). av_iter yields the prev group's AV matmuls one at a
                time, interleaved finely so the PE always has work while
                Activation runs exp."""
                qlo = g * 512
                ET_g = E_pool.tile([P, NB, 512], f8, tag="ET")
                aug = st_pool.tile([1, 2, 512], f8, tag="aug")
                # one PSUM bank serves both the d row (early) and the Z
                # accumulation (later; its start=True reset begins a new
                # group). Z lands duplicated on 128 partitions (the DR
                # stationary must look like a full [c, 2, 128] weight block);
                # row 0 is used.
                Z_ps = sm_ps_pool.tile([P, 512], f32, tag="Z")

                # Z emission runs two tiles behind exp so the PE doesn't
                # stall waiting for the Activation engine at each Z matmul
                zq = []
                nz = [0]

                def z_push(j):
                    zq.append(j)
                    if len(zq) > 2:
                        z_emit(zq.pop(0), False)

                def z_emit(j, last):
                    nc.tensor.matmul(
                        Z_ps[:],
                        ones8[:],
                        ET_g[:, 2 * j : 2 * j + 2, :],
                        start=(nz[0] == 0),
                        stop=last,
                        perf_mode=DR,
                    )
                    nz[0] += 1

                # bias row: d_q = ||h_q||^2 from squared hT columns of this
                # group (needs hT chunk g), summed over partitions by an
                # f32r ones-matmul into the (shared) Z bank
                yield g
                sq = E_pool.tile([P, DB, 512], f32r, tag="sq")
                for dc in range(DB):
                    nc.vector.tensor_mul(
                        sq[:, dc, :],
                        hT[:, dc, qlo : qlo + 512],
                        hT[:, dc, qlo : qlo + 512],
                    )
                for dc in range(DB):
                    nc.tensor.matmul(
                        Z_ps[0:1, :],
                        ones_r[:, 0:1],
                        sq[:, dc, :],
                        start=(dc == 0),
                        stop=(dc == DB - 1),
                    )
                nc.gpsimd.memset(aug[:], 0.0)
                nc.vector.tensor_scalar(
                    aug[0:1, 0, :], Z_ps[0:1, :], -1.0 / 64.0, None, op0=ALU.mult
                )
                # finish prev group's reciprocal (PE transpose was kept off
                # its critical path), then overlap some of its AV work here
                if finalize_prev[0] is not None:
                    finalize_prev[0]()
                    finalize_prev[0] = None
                pull(av_iter, 12)

                for j in range(NT):
                    if j // 2 > g:
                        yield j // 2
                    s_ps = sc_ps_pool.tile([P, 2, 512], f32, tag="s")
                    score_tile_mms(s_ps, j, qlo, g, aug)
                    nc.scalar.activation(
                        ET_g[:, 2 * j : 2 * j + 2, :],
                        s_ps[:],
                        AF.Exp,
                        bias=0.0,
                        scale=1.0,
                    )
                    z_push(j)
                    pull(av_iter, 4)
                z_emit(zq.pop(0), False)
                z_emit(zq.pop(0), True)
                pull(av_iter, 1 << 20)

                # rowsum -> per-q reciprocal columns [P, 4]; the PE transpose
                # is deferred into the next group's stream (finalize)
                zrow = st_pool.tile([1, 512], f32, tag="zrow")
                nc.vector.tensor_scalar(
                    zrow[:], Z_ps[0:1, :], 1.0, None, op0=ALU.add
                )
                rec_row = st_pool.tile([1, 512], f16, tag="rrow")
                with nc.allow_low_precision(reason="f16 recip, tol 2e-2"):
                    nc.vector.reciprocal(rec_row[:], zrow[:])
                resh = st_pool.tile([4, P], f16, tag="resh")
                nc.sync.dma_start(resh[:], rec_row[0:1, :])
                recip_cols = st_pool.tile([P, 4], f32, tag="rc")

                def finalize():
                    rT_ps = sm_ps_pool.tile([P, DB, P], f16, tag="tr")
                    nc.tensor.transpose(
                        rT_ps[:, 0, 0:4], resh[:], ident_h[0:4, 0:4]
                    )
                    nc.vector.tensor_copy(recip_cols[:], rT_ps[:, 0, 0:4])

                finalize_prev[0] = finalize
                state[g] = (ET_g, recip_cols)

            def av_stream(g):
                """Yield once per AV matmul of group g; scale + store emitted
                inline at each block end."""
                ET_g, recip_cols = state[g]
                for qi in range(4):
                    Q = 4 * g + qi
                    o_ps = o_ps_pool.tile([P, D], f32, tag="o")
                    for k in range(NB // 2):
                        nc.tensor.matmul(
                            o_ps[:],
                            ET_g[:, 2 * k : 2 * k + 2, qi * P : (qi + 1) * P],
                            x8[:, 2 * k : 2 * k + 2, :],
                            start=(k == 0),
                            stop=False,
                            perf_mode=DR,
                        )
                        yield
                    # diag contribution at f16 precision closes the group
                    # (kept last: its inputs are ready from the start, and a
                    # dep-scheduled hoist of an open 'start' would wedge the
                    # o_ps bank rotation)
                    nc.tensor.matmul(
                        o_ps[:], ident_h[:], x_sb[:, Q, :], start=False, stop=True
                    )
                    yield
                    out_sb = out_pool.tile([P, D], f32, tag="out")
                    nc.vector.tensor_scalar(
                        out_sb[:], o_ps[:], recip_cols[:, qi : qi + 1], None,
                        op0=ALU.mult,
                    )
                    nc.sync.dma_start(OUT.ap()[Q * P : (Q + 1) * P, :], out_sb[:])
                state.pop(g)

            # ---- driver: phase 1 chunks interleaved with group 0, then the
            # remaining groups each interleaving the previous group's AV ----
            finalize_prev = [None]
            g0 = scores_stage(0, None)
            need = next(g0)
            for c in range(N // 512):
                p1_chunk(c)
                while need is not None and need <= c:
                    need = next(g0, None)
            assert need is None
            p1.close()
            for g in range(1, G):
                for _ in scores_stage(g, av_stream(g - 1)):
                    pass
            finalize_prev[0]()
            finalize_prev[0] = None
            pull(av_stream(G - 1), 1 << 20)
            if reps != 1:
                p2.close()

    nc.compile()
    return nc


def _get_nc(reps: int = 1):
    key = ("nc", reps)
    if key not in _CACHE:
        _CACHE[key] = _build(reps)
    return _CACHE[key]


def kernel(x_resting: np.ndarray, W: np.ndarray, b: np.ndarray) -> np.ndarray:
    from concourse.bass_utils import run_bass_kernel_spmd

    nc = _get_nc()
    in_maps = [
        {
            "x": np.ascontiguousarray(x_resting, dtype=np.float32),
            "w": np.ascontiguousarray(W[c], dtype=np.float32),
            "b": np.ascontiguousarray(b[c].reshape(D, 1), dtype=np.float32),
        }
        for c in range(N_CORES)
    ]
    res = run_bass_kernel_spmd(nc, in_maps, list(range(N_CORES)))
    return np.concatenate([res.results[c]["out"] for c in range(N_CORES)], axis=1)
